# revision 3
# baseline (speedup 1.0000x reference)
"""CondConv (per-sample routed 3x3 conv) on 8 Trainium2 NeuronCores.

Reference computation (all fp32):
    gap     = mean(x, axis=(2,3))                    [B, CIN]
    routing = sigmoid(gap @ W_att.T + b_att)         [B, E]
    ker     = einsum('be,eoihw->boihw', routing, convs)
    out[b]  = conv2d(x[b], ker[b], stride 1, pad 1)  [B, COUT, 56, 56]

Sharding (B=32, COUT=256 across 8 cores): 4 core-pairs; pair p owns
samples 8p..8p+7 (batch data-parallel), and within a pair each core
computes one half of COUT (128 channels).

v3, informed by HW traces:
  - conv matmuls stay fp32r (measured 209ns/matmul inter-arrival vs
    bf16's 230ns for this strided rhs shape);
  - the expert bank is bf16 (halves its DMA + SBUF; the DVE mix reads
    bf16 and accumulates into an fp32r kt via mixed-dtype STT);
  - DMA is split over both HWDGE rings: x input on the SP ring,
    expert bank + output stores on the Activation ring (the baseline
    pushed 50MB through one ring at ~185GB/s, 96% busy);
  - a post-schedule pass deletes redundant InstLdweights (the tile
    scheduler emits one per matmul; 6 of every 7 reload identical
    weights — each reload occupies the PE for ~187ns);
  - the final shift pass of each conv and its PSUM drains run in
    reverse tile order so banks free in the order the next sample's
    forward-order matmuls consume them; drains alternate DVE/ScalarE.
"""

import numpy as np

B, CIN, H, W = 32, 256, 56, 56
COUT, KK, E = 256, 3, 8
HP, WP = H + 2, W + 2          # zero-padded input plane
PHW = HP * WP                  # 3364
NSH = KK * KK                  # 9 shifts
CHUNKS = 2                     # CIN = 2 * 128
MHALF = COUT // 2              # couts per core
ROWS_PER_TILE = 8              # output rows per matmul tile
NTILES = H // ROWS_PER_TILE    # 7
NFREE = ROWS_PER_TILE * W      # 448
NCORES = 8
SAMPLES_PER_CORE = B // (NCORES // 2)  # 8

_cached = {}


def _dedup_ldweights(nc):
    """Remove InstLdweights that reload the exact weights already resident
    in the PE array. The tile scheduler splits every InstMatmult into
    InstLdweights + InstMatmult(ldweights=False); consecutive matmuls in a
    (chunk, shift) group share one lhsT, so 6 of 7 loads are redundant.
    Only loads with no waits/updates/descendants are dropped; any other PE
    instruction except matmuls resets the tracked weights."""
    removed = 0
    for fn in nc.m.functions:
        for bb in fn.blocks:
            il = bb.instructions
            keep = []
            last_w = None
            changed = False
            for inst in il:
                nm = type(inst).__name__
                if nm == "InstLdweights":
                    si = inst.sync_info
                    key = (str(inst.ins[0]), str(inst.tile_position),
                           str(inst.tile_size), str(inst.perf_mode),
                           str(inst.is_transpose))
                    clean = (inst.descendants is None and
                             (si is None or (not si.on_wait and not si.on_update)))
                    if key == last_w and clean:
                        removed += 1
                        changed = True
                        continue
                    last_w = key
                elif nm == "InstMatmult":
                    pass          # matmuls don't clobber loaded weights
                elif str(getattr(inst, "engine", "")) == "EngineType.PE" and \
                        nm not in ("InstEventSemaphore",):
                    last_w = None  # conservative reset on other PE ops
                keep.append(inst)
            if changed:
                bb.instructions = keep
    return removed


def _build_program():
    import concourse.bacc as bacc
    import concourse.bass_isa as bass_isa
    import concourse.mybir as mybir
    from concourse.tile import TileContext

    f32 = mybir.dt.float32
    f32r = mybir.dt.float32r
    bf16 = mybir.dt.bfloat16
    Alu = mybir.AluOpType
    Act = mybir.ActivationFunctionType

    nc = bacc.Bacc(None, target_bir_lowering=False)

    xpad_d = nc.declare_dram_parameter(
        "xpad", [SAMPLES_PER_CORE, CHUNKS, 128, PHW], f32r, isOutput=False)
    convsT_d = nc.declare_dram_parameter(
        "convsT", [E, CHUNKS, 128, NSH * 128], bf16, isOutput=False)
    watt_d = nc.declare_dram_parameter("watt", [CHUNKS, 128, E], f32, isOutput=False)
    battb_d = nc.declare_dram_parameter("battb", [128, E], f32, isOutput=False)
    out_d = nc.declare_dram_parameter(
        "out", [SAMPLES_PER_CORE, MHALF, H, W], f32, isOutput=True)

    with TileContext(nc) as tc:
        with (
            tc.tile_pool(name="resident", bufs=1) as res_pool,
            tc.tile_pool(name="xp", bufs=3) as xp_pool,
            tc.tile_pool(name="kt", bufs=3) as kt_pool,
            tc.tile_pool(name="small", bufs=3) as small_pool,
            tc.tile_pool(name="outsb", bufs=4) as out_pool,
            tc.tile_pool(name="cpsum", bufs=1, space="PSUM") as cps_pool,
        ):
            # ---- small resident tiles -------------------------------------
            watt_sb = []
            for c in range(CHUNKS):
                t = res_pool.tile([128, E], f32, name=f"watt{c}", tag=f"watt{c}")
                nc.sync.dma_start(out=t[:], in_=watt_d[c])
                watt_sb.append(t)
            battb_sb = res_pool.tile([128, E], f32, name="battb", tag="battb")
            nc.sync.dma_start(out=battb_sb[:], in_=battb_d[:])
            # broadcast routing weights: scal[:, 8*b+e] = r_be on every partition
            scal_sb = res_pool.tile([128, SAMPLES_PER_CORE * E], f32,
                                    name="scal", tag="scal")

            convsT_sb = [[None] * CHUNKS for _ in range(E)]

            def emit_load_dma(b, split4):
                """DMA fp32 padded input for sample b on the SP ring.
                4-way split for the pipeline-fill samples (lower latency);
                one DMA per chunk afterwards (fewer descriptors)."""
                xp = []
                quarter = PHW // 4  # 841
                for c in range(CHUNKS):
                    t = xp_pool.tile([128, PHW], f32r, name=f"xp{c}", tag=f"xp{c}")
                    if split4:
                        for j in range(4):
                            sl = slice(j * quarter, (j + 1) * quarter)
                            nc.sync.dma_start(out=t[:, sl], in_=xpad_d[b, c, :, sl])
                    else:
                        nc.sync.dma_start(out=t[:], in_=xpad_d[b, c])
                    xp.append(t)
                return xp

            def emit_load_gap(xp):
                """GAP/rounding pass: ScalarE in-place Copy rounds fp32 ->
                fp32r (walrus requires fp32r matmul inputs to come from a
                rounding producer) and its accum_out yields the GAP row
                sums. Split in halves to overlap the input DMA."""
                gq = []
                half = PHW // 2
                for c in range(CHUNKS):
                    for h in range(2):
                        sl = slice(h * half, (h + 1) * half)
                        g = small_pool.tile([128, 1], f32, name=f"gh{c}_{h}",
                                            tag=f"gh{c}_{h}")
                        nc.scalar.activation(out=xp[c][:, sl], in_=xp[c][:, sl],
                                             func=Act.Copy, accum_out=g[:])
                        gq.append(g)
                return gq

            def emit_load(b, split4=False):
                xp = emit_load_dma(b, split4)
                return xp, emit_load_gap(xp)

            def emit_routing(b, gs):
                """Routing for sample b on DVE/GPSIMD/ScalarE only.

                logits[e] = sum_cin gap[cin] * W_att[e,cin] / 3136 + b_att[e]
                (the 1/3136 is folded into watt host-side).
                """
                gsum = []
                for c in range(CHUNKS):
                    g = small_pool.tile([128, 1], f32, name=f"gs{c}", tag=f"gs{c}")
                    nc.vector.tensor_add(out=g[:], in0=gs[2 * c][:],
                                         in1=gs[2 * c + 1][:])
                    gsum.append(g)
                t0 = small_pool.tile([128, E], f32, name="t0", tag="t0")
                nc.vector.tensor_scalar_mul(out=t0[:], in0=watt_sb[0][:],
                                            scalar1=gsum[0][:, 0:1])
                t1 = small_pool.tile([128, E], f32, name="t1", tag="t1")
                nc.vector.scalar_tensor_tensor(
                    out=t1[:], in0=watt_sb[1][:], scalar=gsum[1][:, 0:1],
                    in1=t0[:], op0=Alu.mult, op1=Alu.add)
                red = small_pool.tile([128, E], f32, name="red", tag="red")
                nc.gpsimd.partition_all_reduce(red[:], t1[:], channels=128,
                                               reduce_op=bass_isa.ReduceOp.add)
                red2 = small_pool.tile([128, E], f32, name="red2", tag="red2")
                nc.vector.tensor_add(out=red2[:], in0=red[:], in1=battb_sb[:])
                nc.scalar.activation(out=scal_sb[:, b * E:(b + 1) * E],
                                     in_=red2[:], func=Act.Sigmoid)

            def emit_mix_chunk(b, c):
                """Mix chunk c of sample b's kernel on VectorE: bf16 expert
                tiles scaled-accumulated into an fp32r kt.
                kerT[c][cin, s*128+m] = sum_e r_be * convsT[e][c][cin, s*128+m]
                """
                k = kt_pool.tile([128, NSH * 128], f32r, name=f"kt{c}", tag=f"kt{c}")
                nc.vector.tensor_scalar_mul(
                    out=k[:], in0=convsT_sb[0][c][:],
                    scalar1=scal_sb[:, b * E:b * E + 1])
                for e in range(1, E):
                    nc.vector.scalar_tensor_tensor(
                        out=k[:], in0=convsT_sb[e][c][:],
                        scalar=scal_sb[:, b * E + e:b * E + e + 1],
                        in1=k[:], op0=Alu.mult, op1=Alu.add)
                return k

            def emit_route_mix(b, gs):
                emit_routing(b, gs)
                return [emit_mix_chunk(b, c) for c in range(CHUNKS)]

            def emit_conv(b, xp, kt):
                """Conv for sample b: accumulate 2c*9shift into 7 PSUM tiles.
                Final pass + drains in reverse tile order (see module doc)."""
                cps = [cps_pool.tile([128, NFREE], f32, name=f"cps{n}",
                                     tag=f"cps{n}", bufs=2 if n == 0 else 1)
                       for n in range(NTILES)]
                for c in range(CHUNKS):
                    x3 = xp[c].rearrange("p (r q) -> p r q", q=WP)
                    for s in range(NSH):
                        dh, dw = s // KK, s % KK
                        lhsT = kt[c][:, s * 128:(s + 1) * 128]
                        first = (c == 0 and s == 0)
                        last = (c == CHUNKS - 1 and s == NSH - 1)
                        order = range(NTILES - 1, -1, -1) if last else range(NTILES)
                        for n in order:
                            rhs = x3[:, n * ROWS_PER_TILE + dh:
                                     n * ROWS_PER_TILE + dh + ROWS_PER_TILE,
                                     dw:dw + W]
                            nc.tensor.matmul(cps[n][:], lhsT, rhs,
                                             start=first, stop=last)
                # drains: reverse order, alternating DVE/ScalarE; the output
                # stores ride the Activation HWDGE ring
                for n in range(NTILES - 1, -1, -1):
                    o = out_pool.tile([128, NFREE], f32, name="osb", tag="osb")
                    if n % 2 == 0:
                        nc.vector.tensor_copy(out=o[:], in_=cps[n][:])
                    else:
                        nc.scalar.activation(out=o[:], in_=cps[n][:], func=Act.Copy)
                    nc.scalar.dma_start(
                        out=out_d[b, :, n * ROWS_PER_TILE:(n + 1) * ROWS_PER_TILE, :],
                        in_=o[:])

            # ---- software-pipelined emission ------------------------------
            S = SAMPLES_PER_CORE

            def emit_bank_chunk(c):
                """Expert bank chunk on the Activation HWDGE ring (bf16)."""
                for e in range(E):
                    t = res_pool.tile([128, NSH * 128], bf16,
                                      name=f"cv_{e}_{c}", tag=f"cv_{e}_{c}")
                    nc.scalar.dma_start(out=t[:], in_=convsT_d[e, c])
                    convsT_sb[e][c] = t

            emit_bank_chunk(0)
            loads = {0: emit_load(0, split4=True)}
            emit_routing(0, loads[0][1])
            kt0c0 = emit_mix_chunk(0, 0)
            emit_bank_chunk(1)
            kt0c1 = emit_mix_chunk(0, 1)
            kts = {0: [kt0c0, kt0c1]}
            loads[1] = emit_load(1, split4=True)
            emit_routing(1, loads[1][1])
            kt1c0 = emit_mix_chunk(1, 0)
            loads[2] = emit_load(2)
            emit_conv(0, loads.pop(0)[0], kts.pop(0))
            kts[1] = [kt1c0, emit_mix_chunk(1, 1)]
            for b in range(1, S):
                if b + 1 < S and b + 1 not in kts:
                    kts[b + 1] = emit_route_mix(b + 1, loads[b + 1][1])
                if b + 2 < S:
                    loads[b + 2] = emit_load(b + 2)
                emit_conv(b, loads.pop(b)[0], kts.pop(b))

    _dedup_ldweights(nc)
    nc.compile()
    return nc


def _prep_core_inputs(x, convs, W_att, b_att):
    """Host-side shard/layout prep. Returns list of 8 per-core input dicts."""
    import ml_dtypes
    f32 = np.float32
    bf16 = ml_dtypes.bfloat16
    # padded fp32 input, cin split into 2 chunks of 128
    xpad = np.zeros((B, CHUNKS, 128, HP, WP), dtype=f32)
    xpad[:, :, :, 1:H + 1, 1:W + 1] = np.ascontiguousarray(x, dtype=f32).reshape(
        B, CHUNKS, 128, H, W)
    xpad = xpad.reshape(B, CHUNKS, 128, PHW)

    # convsT[half][e, c, cin, s*128 + m] = convs[e, half*128+m, c*128+cin, kh, kw]
    cv = np.ascontiguousarray(convs, dtype=f32).reshape(E, 2, MHALF, CHUNKS, 128, NSH)
    convsT_halves = [
        np.ascontiguousarray(cv[:, h].transpose(0, 2, 3, 4, 1).reshape(
            E, CHUNKS, 128, NSH * 128)).astype(bf16)
        for h in range(2)
    ]

    watt = np.ascontiguousarray(
        (np.asarray(W_att, dtype=f32).T / f32(H * W)).reshape(CHUNKS, 128, E))
    battb = np.ascontiguousarray(
        np.broadcast_to(np.asarray(b_att, dtype=f32), (128, E)))

    in_maps = []
    for k in range(NCORES):
        pair, half = k // 2, k % 2
        sl = slice(pair * SAMPLES_PER_CORE, (pair + 1) * SAMPLES_PER_CORE)
        in_maps.append({
            "xpad": np.ascontiguousarray(xpad[sl]),
            "convsT": convsT_halves[half],
            "watt": watt,
            "battb": battb,
        })
    return in_maps


def _assemble_output(results):
    out = np.empty((B, COUT, H, W), dtype=np.float32)
    for k in range(NCORES):
        pair, half = k // 2, k % 2
        sl = slice(pair * SAMPLES_PER_CORE, (pair + 1) * SAMPLES_PER_CORE)
        out[sl, half * MHALF:(half + 1) * MHALF] = results[k]["out"]
    return out


def kernel(x, convs, W_att, b_att):
    from concourse.bass_utils import run_bass_kernel_spmd

    if "nc" not in _cached:
        _cached["nc"] = _build_program()
    in_maps = _prep_core_inputs(x, convs, W_att, b_att)
    res = run_bass_kernel_spmd(_cached["nc"], in_maps, core_ids=list(range(NCORES)))
    return _assemble_output(res.results)


# revision 4
# speedup vs baseline: 1.0396x; 1.0396x over previous
"""CondConv (per-sample routed 3x3 conv) on 8 Trainium2 NeuronCores.

Reference computation (all fp32):
    gap     = mean(x, axis=(2,3))                    [B, CIN]
    routing = sigmoid(gap @ W_att.T + b_att)         [B, E]
    ker     = einsum('be,eoihw->boihw', routing, convs)
    out[b]  = conv2d(x[b], ker[b], stride 1, pad 1)  [B, COUT, 56, 56]

Sharding (B=32, COUT=256 across 8 cores): 4 core-pairs; pair p owns
samples 8p..8p+7 (batch data-parallel), and within a pair each core
computes one half of COUT (128 channels).

v5. HW traces showed the baseline was DMA-throughput-bound: ~50MB
(fp32 x in + fp32 out + fp32 bank) through one HWDGE ring at its
~180GB/s practical ceiling = the whole 280us. So:
  - everything on the wire is bf16 (x, expert bank, outputs): ~25MB
    total, comfortably under the PE time; the 2e-2 rel-err gate
    leaves bf16 ~5x margin (measured 3.8e-3 end to end);
  - DMA is balanced across BOTH HWDGE rings (SP + Activation): x
    chunk0/even bank/even out-tiles on SP, chunk1/odd on Activation;
  - a post-schedule pass deletes redundant InstLdweights (the tile
    scheduler emits one per matmul; 6 of 7 reload identical weights,
    each occupying the PE ~116ns);
  - per-sample kernel mix: DVE accumulates experts 0-5 (bf16 STT
    chain), ScalarE produces scaled copies of experts 6-7
    (activation Copy with scale=routing), DVE folds them in;
  - the first and last shift passes of each conv and the drains run
    in reverse tile order so PSUM banks free exactly in the order
    the next sample's matmuls consume them; drains alternate
    DVE/ScalarE.
"""

import numpy as np

B, CIN, H, W = 32, 256, 56, 56
COUT, KK, E = 256, 3, 8
HP, WP = H + 2, W + 2          # zero-padded input plane
PHW = HP * WP                  # 3364
NSH = KK * KK                  # 9 shifts
CHUNKS = 2                     # CIN = 2 * 128
MHALF = COUT // 2              # couts per core
ROWS_PER_TILE = 8              # output rows per matmul tile
NTILES = H // ROWS_PER_TILE    # 7
NFREE = ROWS_PER_TILE * W      # 448
NCORES = 8
SAMPLES_PER_CORE = B // (NCORES // 2)  # 8

_cached = {}


def _dedup_ldweights(nc):
    """Remove InstLdweights that reload the exact weights already resident
    in the PE array. The tile scheduler splits every bf16 InstMatmult into
    InstLdweights + InstMatmult(ldweights=False); consecutive matmuls in a
    (chunk, shift) group share one lhsT, so 6 of 7 loads are redundant.
    Only loads with no waits/updates/descendants are dropped; any other PE
    instruction except matmuls/event-semaphores resets the tracked key."""
    removed = 0
    for fn in nc.m.functions:
        for bb in fn.blocks:
            il = bb.instructions
            keep = []
            last_w = None
            changed = False
            for inst in il:
                nm = type(inst).__name__
                if nm == "InstLdweights":
                    si = inst.sync_info
                    key = (str(inst.ins[0]), str(inst.tile_position),
                           str(inst.tile_size), str(inst.perf_mode),
                           str(inst.is_transpose))
                    clean = (inst.descendants is None and
                             (si is None or (not si.on_wait and not si.on_update)))
                    if key == last_w and clean:
                        removed += 1
                        changed = True
                        continue
                    last_w = key
                elif nm == "InstMatmult":
                    pass          # matmuls don't clobber loaded weights
                elif str(getattr(inst, "engine", "")) == "EngineType.PE" and \
                        nm not in ("InstEventSemaphore",):
                    last_w = None  # conservative reset on other PE ops
                keep.append(inst)
            if changed:
                bb.instructions = keep
    return removed


def _build_program():
    import concourse.bacc as bacc
    import concourse.bass_isa as bass_isa
    import concourse.mybir as mybir
    from concourse.tile import TileContext

    f32 = mybir.dt.float32
    bf16 = mybir.dt.bfloat16
    Alu = mybir.AluOpType
    Act = mybir.ActivationFunctionType

    nc = bacc.Bacc(None, target_bir_lowering=False)

    xpad_d = nc.declare_dram_parameter(
        "xpad", [SAMPLES_PER_CORE, CHUNKS, 128, PHW], bf16, isOutput=False)
    convsT_d = nc.declare_dram_parameter(
        "convsT", [E, CHUNKS, 128, NSH * 128], bf16, isOutput=False)
    watt_d = nc.declare_dram_parameter("watt", [CHUNKS, 128, E], f32, isOutput=False)
    battb_d = nc.declare_dram_parameter("battb", [128, E], f32, isOutput=False)
    out_d = nc.declare_dram_parameter(
        "out", [SAMPLES_PER_CORE, MHALF, H, W], bf16, isOutput=True)

    with TileContext(nc) as tc:
        with (
            tc.tile_pool(name="resident", bufs=1) as res_pool,
            tc.tile_pool(name="xp", bufs=3) as xp_pool,
            tc.tile_pool(name="kt", bufs=3) as kt_pool,
            tc.tile_pool(name="amix", bufs=3) as amix_pool,
            tc.tile_pool(name="small", bufs=3) as small_pool,
            tc.tile_pool(name="outsb", bufs=4) as out_pool,
            tc.tile_pool(name="cpsum", bufs=1, space="PSUM") as cps_pool,
        ):
            # ---- small resident tiles -------------------------------------
            watt_sb = []
            for c in range(CHUNKS):
                t = res_pool.tile([128, E], f32, name=f"watt{c}", tag=f"watt{c}")
                nc.sync.dma_start(out=t[:], in_=watt_d[c])
                watt_sb.append(t)
            battb_sb = res_pool.tile([128, E], f32, name="battb", tag="battb")
            nc.sync.dma_start(out=battb_sb[:], in_=battb_d[:])
            # broadcast routing weights: scal[:, 8*b+e] = r_be on every partition
            scal_sb = res_pool.tile([128, SAMPLES_PER_CORE * E], f32,
                                    name="scal", tag="scal")

            convsT_sb = [[None] * CHUNKS for _ in range(E)]

            def emit_load_dma(b, split4):
                """DMA bf16 padded input for sample b: chunk0 on the SP ring,
                chunk1 on the Activation ring. 4-way split for the
                pipeline-fill samples (latency), whole-chunk afterwards."""
                xp = []
                quarter = PHW // 4  # 841
                for c in range(CHUNKS):
                    eng = nc.sync if c == 0 else nc.scalar
                    t = xp_pool.tile([128, PHW], bf16, name=f"xp{c}", tag=f"xp{c}")
                    if split4:
                        for j in range(4):
                            sl = slice(j * quarter, (j + 1) * quarter)
                            eng.dma_start(out=t[:, sl], in_=xpad_d[b, c, :, sl])
                    else:
                        eng.dma_start(out=t[:], in_=xpad_d[b, c])
                    xp.append(t)
                return xp

            def emit_load_gap(xp):
                """GAP pass: ScalarE in-place Copy whose fp32 accum_out
                yields the per-cin row sums. Split in halves so it starts
                while the input DMA is still landing."""
                gq = []
                half = PHW // 2
                for c in range(CHUNKS):
                    for h in range(2):
                        sl = slice(h * half, (h + 1) * half)
                        g = small_pool.tile([128, 1], f32, name=f"gh{c}_{h}",
                                            tag=f"gh{c}_{h}")
                        nc.scalar.activation(out=xp[c][:, sl], in_=xp[c][:, sl],
                                             func=Act.Copy, accum_out=g[:])
                        gq.append(g)
                return gq

            def emit_load(b, split4=False):
                xp = emit_load_dma(b, split4)
                return xp, emit_load_gap(xp)

            def emit_routing(b, gs):
                """Routing for sample b on DVE/GPSIMD/ScalarE only.

                logits[e] = sum_cin gap[cin] * W_att[e,cin] / 3136 + b_att[e]
                (the 1/3136 is folded into watt host-side).
                """
                gsum = []
                for c in range(CHUNKS):
                    g = small_pool.tile([128, 1], f32, name=f"gs{c}", tag=f"gs{c}")
                    nc.vector.tensor_add(out=g[:], in0=gs[2 * c][:],
                                         in1=gs[2 * c + 1][:])
                    gsum.append(g)
                t0 = small_pool.tile([128, E], f32, name="t0", tag="t0")
                nc.vector.tensor_scalar_mul(out=t0[:], in0=watt_sb[0][:],
                                            scalar1=gsum[0][:, 0:1])
                t1 = small_pool.tile([128, E], f32, name="t1", tag="t1")
                nc.vector.scalar_tensor_tensor(
                    out=t1[:], in0=watt_sb[1][:], scalar=gsum[1][:, 0:1],
                    in1=t0[:], op0=Alu.mult, op1=Alu.add)
                red = small_pool.tile([128, E], f32, name="red", tag="red")
                nc.gpsimd.partition_all_reduce(red[:], t1[:], channels=128,
                                               reduce_op=bass_isa.ReduceOp.add)
                red2 = small_pool.tile([128, E], f32, name="red2", tag="red2")
                nc.vector.tensor_add(out=red2[:], in0=red[:], in1=battb_sb[:])
                nc.scalar.activation(out=scal_sb[:, b * E:(b + 1) * E],
                                     in_=red2[:], func=Act.Sigmoid)

            def emit_mix_chunk(b, c):
                """Mix chunk c of sample b's kernel, all bf16:
                kerT[c][cin, s*128+m] = sum_e r_be * convsT[e][c][cin, s*128+m]
                DVE: e0 tensor_scalar (4x mode) + e1..e5 STT chain.
                ScalarE: scaled copies of e6, e7 (activation scale=r).
                DVE folds: kt += (a6 + a7).
                """
                sc = lambda e: scal_sb[:, b * E + e:b * E + e + 1]
                a6 = amix_pool.tile([128, NSH * 128], bf16, name=f"a6_{c}",
                                    tag=f"a6_{c}")
                nc.scalar.activation(out=a6[:], in_=convsT_sb[6][c][:],
                                     func=Act.Copy, scale=sc(6))
                a7 = amix_pool.tile([128, NSH * 128], bf16, name=f"a7_{c}",
                                    tag=f"a7_{c}")
                nc.scalar.activation(out=a7[:], in_=convsT_sb[7][c][:],
                                     func=Act.Copy, scale=sc(7))
                k = kt_pool.tile([128, NSH * 128], bf16, name=f"kt{c}", tag=f"kt{c}")
                nc.vector.tensor_scalar_mul(out=k[:], in0=convsT_sb[0][c][:],
                                            scalar1=sc(0))
                for e in range(1, 6):
                    nc.vector.scalar_tensor_tensor(
                        out=k[:], in0=convsT_sb[e][c][:], scalar=sc(e),
                        in1=k[:], op0=Alu.mult, op1=Alu.add)
                u = amix_pool.tile([128, NSH * 128], bf16, name=f"u{c}", tag=f"u{c}")
                nc.vector.tensor_add(out=u[:], in0=a6[:], in1=a7[:])
                nc.vector.tensor_add(out=k[:], in0=k[:], in1=u[:])
                return k

            def emit_route_mix(b, gs):
                emit_routing(b, gs)
                return [emit_mix_chunk(b, c) for c in range(CHUNKS)]

            def emit_conv(b, xp, kt):
                """Conv for sample b: accumulate 2c*9shift into 7 PSUM tiles.
                First and last passes + drains run in reverse tile order so
                PSUM banks recycle in consumption order (see module doc)."""
                cps = [cps_pool.tile([128, NFREE], f32, name=f"cps{n}",
                                     tag=f"cps{n}", bufs=2 if n == 0 else 1)
                       for n in range(NTILES)]
                for c in range(CHUNKS):
                    x3 = xp[c].rearrange("p (r q) -> p r q", q=WP)
                    for s in range(NSH):
                        dh, dw = s // KK, s % KK
                        lhsT = kt[c][:, s * 128:(s + 1) * 128]
                        first = (c == 0 and s == 0)
                        last = (c == CHUNKS - 1 and s == NSH - 1)
                        order = (range(NTILES - 1, -1, -1) if (first or last)
                                 else range(NTILES))
                        for n in order:
                            rhs = x3[:, n * ROWS_PER_TILE + dh:
                                     n * ROWS_PER_TILE + dh + ROWS_PER_TILE,
                                     dw:dw + W]
                            nc.tensor.matmul(cps[n][:], lhsT, rhs,
                                             start=first, stop=last)
                # drains in reverse order, alternating DVE/ScalarE; out DMAs
                # alternate SP/Activation rings (bf16 stores)
                for n in range(NTILES - 1, -1, -1):
                    o = out_pool.tile([128, NFREE], bf16, name="osb", tag="osb")
                    if n % 2 == 0:
                        nc.vector.tensor_copy(out=o[:], in_=cps[n][:])
                        deng = nc.sync
                    else:
                        nc.scalar.activation(out=o[:], in_=cps[n][:], func=Act.Copy)
                        deng = nc.scalar
                    deng.dma_start(
                        out=out_d[b, :, n * ROWS_PER_TILE:(n + 1) * ROWS_PER_TILE, :],
                        in_=o[:])

            # ---- software-pipelined emission ------------------------------
            S = SAMPLES_PER_CORE

            def emit_bank_chunk(c):
                """Expert bank chunk, tiles alternating SP/Activation rings."""
                for e in range(E):
                    t = res_pool.tile([128, NSH * 128], bf16,
                                      name=f"cv_{e}_{c}", tag=f"cv_{e}_{c}")
                    eng = nc.sync if e % 2 == 0 else nc.scalar
                    eng.dma_start(out=t[:], in_=convsT_d[e, c])
                    convsT_sb[e][c] = t

            emit_bank_chunk(0)
            loads = {0: emit_load(0, split4=True)}
            emit_routing(0, loads[0][1])
            kt0c0 = emit_mix_chunk(0, 0)
            emit_bank_chunk(1)
            kt0c1 = emit_mix_chunk(0, 1)
            kts = {0: [kt0c0, kt0c1]}
            loads[1] = emit_load(1, split4=True)
            emit_routing(1, loads[1][1])
            kt1c0 = emit_mix_chunk(1, 0)
            loads[2] = emit_load(2)
            emit_conv(0, loads.pop(0)[0], kts.pop(0))
            kts[1] = [kt1c0, emit_mix_chunk(1, 1)]
            for b in range(1, S):
                if b + 1 < S and b + 1 not in kts:
                    kts[b + 1] = emit_route_mix(b + 1, loads[b + 1][1])
                if b + 2 < S:
                    loads[b + 2] = emit_load(b + 2)
                emit_conv(b, loads.pop(b)[0], kts.pop(b))

    _dedup_ldweights(nc)
    nc.compile()
    return nc


def _prep_core_inputs(x, convs, W_att, b_att):
    """Host-side shard/layout prep. Returns list of 8 per-core input dicts."""
    import ml_dtypes
    f32 = np.float32
    bf16 = ml_dtypes.bfloat16
    # padded bf16 input, cin split into 2 chunks of 128
    xpad = np.zeros((B, CHUNKS, 128, HP, WP), dtype=bf16)
    xpad[:, :, :, 1:H + 1, 1:W + 1] = np.asarray(x, dtype=f32).reshape(
        B, CHUNKS, 128, H, W).astype(bf16)
    xpad = xpad.reshape(B, CHUNKS, 128, PHW)

    # convsT[half][e, c, cin, s*128 + m] = convs[e, half*128+m, c*128+cin, kh, kw]
    cv = np.ascontiguousarray(convs, dtype=f32).reshape(E, 2, MHALF, CHUNKS, 128, NSH)
    convsT_halves = [
        np.ascontiguousarray(cv[:, h].transpose(0, 2, 3, 4, 1).reshape(
            E, CHUNKS, 128, NSH * 128)).astype(bf16)
        for h in range(2)
    ]

    watt = np.ascontiguousarray(
        (np.asarray(W_att, dtype=f32).T / f32(H * W)).reshape(CHUNKS, 128, E))
    battb = np.ascontiguousarray(
        np.broadcast_to(np.asarray(b_att, dtype=f32), (128, E)))

    in_maps = []
    for k in range(NCORES):
        pair, half = k // 2, k % 2
        sl = slice(pair * SAMPLES_PER_CORE, (pair + 1) * SAMPLES_PER_CORE)
        in_maps.append({
            "xpad": np.ascontiguousarray(xpad[sl]),
            "convsT": convsT_halves[half],
            "watt": watt,
            "battb": battb,
        })
    return in_maps


def _assemble_output(results):
    out = np.empty((B, COUT, H, W), dtype=np.float32)
    for k in range(NCORES):
        pair, half = k // 2, k % 2
        sl = slice(pair * SAMPLES_PER_CORE, (pair + 1) * SAMPLES_PER_CORE)
        out[sl, half * MHALF:(half + 1) * MHALF] = np.asarray(
            results[k]["out"], dtype=np.float32)
    return out


def kernel(x, convs, W_att, b_att):
    from concourse.bass_utils import run_bass_kernel_spmd

    if "nc" not in _cached:
        _cached["nc"] = _build_program()
    in_maps = _prep_core_inputs(x, convs, W_att, b_att)
    res = run_bass_kernel_spmd(_cached["nc"], in_maps, core_ids=list(range(NCORES)))
    return _assemble_output(res.results)


# revision 7
# speedup vs baseline: 1.1769x; 1.1320x over previous
"""CondConv (per-sample routed 3x3 conv) on 8 Trainium2 NeuronCores.

Reference computation (all fp32):
    gap     = mean(x, axis=(2,3))                    [B, CIN]
    routing = sigmoid(gap @ W_att.T + b_att)         [B, E]
    ker     = einsum('be,eoihw->boihw', routing, convs)
    out[b]  = conv2d(x[b], ker[b], stride 1, pad 1)  [B, COUT, 56, 56]

Sharding (B=32, COUT=256 across 8 cores): 4 core-pairs; pair p owns
samples 8p..8p+7 (batch data-parallel), and within a pair each core
computes one half of COUT (128 channels).

v5. HW traces showed the baseline was DMA-throughput-bound: ~50MB
(fp32 x in + fp32 out + fp32 bank) through one HWDGE ring at its
~180GB/s practical ceiling = the whole 280us. So:
  - everything on the wire is bf16 (x, expert bank, outputs): ~25MB
    total, comfortably under the PE time; the 2e-2 rel-err gate
    leaves bf16 ~5x margin (measured 3.8e-3 end to end);
  - DMA is balanced across BOTH HWDGE rings (SP + Activation): x
    chunk0/even bank/even out-tiles on SP, chunk1/odd on Activation;
  - a post-schedule pass deletes redundant InstLdweights (the tile
    scheduler emits one per matmul; 6 of 7 reload identical weights,
    each occupying the PE ~116ns);
  - per-sample kernel mix: DVE accumulates experts 0-5 (bf16 STT
    chain), ScalarE produces scaled copies of experts 6-7
    (activation Copy with scale=routing), DVE folds them in;
  - the first and last shift passes of each conv and the drains run
    in reverse tile order so PSUM banks free exactly in the order
    the next sample's matmuls consume them; drains alternate
    DVE/ScalarE.
"""

import numpy as np

B, CIN, H, W = 32, 256, 56, 56
COUT, KK, E = 256, 3, 8
HP, WP = H + 2, W + 2          # zero-padded input plane
PHW = HP * WP                  # 3364
NSH = KK * KK                  # 9 shifts
CHUNKS = 2                     # CIN = 2 * 128
MHALF = COUT // 2              # couts per core
ROWS_PER_TILE = 8              # output rows per matmul tile
NTILES = H // ROWS_PER_TILE    # 7
NFREE = ROWS_PER_TILE * W      # 448
NCORES = 8
SAMPLES_PER_CORE = B // (NCORES // 2)  # 8

_cached = {}


def _dedup_ldweights(nc):
    """Remove InstLdweights that reload the exact weights already resident
    in the PE array. The tile scheduler splits every bf16 InstMatmult into
    InstLdweights + InstMatmult(ldweights=False); consecutive matmuls in a
    (chunk, shift) group share one lhsT, so 6 of 7 loads are redundant.
    Only loads with no waits/updates/descendants are dropped; any other PE
    instruction except matmuls/event-semaphores resets the tracked key."""
    removed = 0
    for fn in nc.m.functions:
        for bb in fn.blocks:
            il = bb.instructions
            keep = []
            last_w = None
            changed = False
            for inst in il:
                nm = type(inst).__name__
                if nm == "InstLdweights":
                    si = inst.sync_info
                    key = (str(inst.ins[0]), str(inst.tile_position),
                           str(inst.tile_size), str(inst.perf_mode),
                           str(inst.is_transpose))
                    clean = (inst.descendants is None and
                             (si is None or (not si.on_wait and not si.on_update)))
                    if key == last_w and clean:
                        removed += 1
                        changed = True
                        continue
                    last_w = key
                elif nm == "InstMatmult":
                    pass          # matmuls don't clobber loaded weights
                elif str(getattr(inst, "engine", "")) == "EngineType.PE" and \
                        nm not in ("InstEventSemaphore",):
                    last_w = None  # conservative reset on other PE ops
                keep.append(inst)
            if changed:
                bb.instructions = keep
    return removed


def _build_program():
    import concourse.bacc as bacc
    import concourse.bass_isa as bass_isa
    import concourse.mybir as mybir
    from concourse.tile import TileContext

    f32 = mybir.dt.float32
    f32r = mybir.dt.float32r
    bf16 = mybir.dt.bfloat16
    Alu = mybir.AluOpType
    Act = mybir.ActivationFunctionType

    nc = bacc.Bacc(None, target_bir_lowering=False)

    xpad_d = nc.declare_dram_parameter(
        "xpad", [SAMPLES_PER_CORE, CHUNKS, 128, PHW], f32r, isOutput=False)
    convsT_d = nc.declare_dram_parameter(
        "convsT", [E, CHUNKS, 128, NSH * 128], bf16, isOutput=False)
    watt_d = nc.declare_dram_parameter("watt", [CHUNKS, 128, E], f32, isOutput=False)
    battb_d = nc.declare_dram_parameter("battb", [128, E], f32, isOutput=False)
    out_d = nc.declare_dram_parameter(
        "out", [SAMPLES_PER_CORE, MHALF, H, W], bf16, isOutput=True)

    with TileContext(nc) as tc:
        with (
            tc.tile_pool(name="resident", bufs=1) as res_pool,
            tc.tile_pool(name="xp", bufs=3) as xp_pool,
            tc.tile_pool(name="kt", bufs=3) as kt_pool,
            tc.tile_pool(name="amix", bufs=2) as amix_pool,
            tc.tile_pool(name="small", bufs=3) as small_pool,
            tc.tile_pool(name="outsb", bufs=6) as out_pool,
            tc.tile_pool(name="cpsum", bufs=1, space="PSUM") as cps_pool,
        ):
            # ---- small resident tiles -------------------------------------
            watt_sb = []
            for c in range(CHUNKS):
                t = res_pool.tile([128, E], f32, name=f"watt{c}", tag=f"watt{c}")
                nc.sync.dma_start(out=t[:], in_=watt_d[c])
                watt_sb.append(t)
            battb_sb = res_pool.tile([128, E], f32, name="battb", tag="battb")
            nc.sync.dma_start(out=battb_sb[:], in_=battb_d[:])
            # broadcast routing weights: scal[:, 8*b+e] = r_be on every partition
            scal_sb = res_pool.tile([128, SAMPLES_PER_CORE * E], f32,
                                    name="scal", tag="scal")

            convsT_sb = [[None] * CHUNKS for _ in range(E)]

            def emit_load_dma(b, split4):
                """DMA bf16 padded input for sample b: chunk0 on the SP ring,
                chunk1 on the Activation ring. 4-way split for the
                pipeline-fill samples (latency), whole-chunk afterwards."""
                xp = []
                quarter = PHW // 4  # 841
                for c in range(CHUNKS):
                    eng = nc.sync if c == 0 else nc.scalar
                    t = xp_pool.tile([128, PHW], f32r, name=f"xp{c}", tag=f"xp{c}")
                    if split4:
                        for j in range(4):
                            sl = slice(j * quarter, (j + 1) * quarter)
                            eng.dma_start(out=t[:, sl], in_=xpad_d[b, c, :, sl])
                    else:
                        eng.dma_start(out=t[:], in_=xpad_d[b, c])
                    xp.append(t)
                return xp

            def emit_load_gap(xp):
                """GAP/rounding pass: ScalarE in-place Copy (fp32->fp32r
                rounding producer for the matmuls) whose fp32 accum_out
                yields the per-cin row sums. Split in halves so it starts
                while the input DMA is still landing."""
                gq = []
                half = PHW // 2
                for c in range(CHUNKS):
                    for h in range(2):
                        sl = slice(h * half, (h + 1) * half)
                        g = small_pool.tile([128, 1], f32, name=f"gh{c}_{h}",
                                            tag=f"gh{c}_{h}")
                        nc.scalar.activation(out=xp[c][:, sl], in_=xp[c][:, sl],
                                             func=Act.Copy, accum_out=g[:])
                        gq.append(g)
                return gq

            def emit_load(b, split4=False):
                xp = emit_load_dma(b, split4)
                return xp, emit_load_gap(xp)

            def emit_routing(b, gs):
                """Routing for sample b on DVE/GPSIMD/ScalarE only.

                logits[e] = sum_cin gap[cin] * W_att[e,cin] / 3136 + b_att[e]
                (the 1/3136 is folded into watt host-side).
                """
                gsum = []
                for c in range(CHUNKS):
                    g = small_pool.tile([128, 1], f32, name=f"gs{c}", tag=f"gs{c}")
                    nc.vector.tensor_add(out=g[:], in0=gs[2 * c][:],
                                         in1=gs[2 * c + 1][:])
                    gsum.append(g)
                t0 = small_pool.tile([128, E], f32, name="t0", tag="t0")
                nc.vector.tensor_scalar_mul(out=t0[:], in0=watt_sb[0][:],
                                            scalar1=gsum[0][:, 0:1])
                t1 = small_pool.tile([128, E], f32, name="t1", tag="t1")
                nc.vector.scalar_tensor_tensor(
                    out=t1[:], in0=watt_sb[1][:], scalar=gsum[1][:, 0:1],
                    in1=t0[:], op0=Alu.mult, op1=Alu.add)
                red = small_pool.tile([128, E], f32, name="red", tag="red")
                nc.gpsimd.partition_all_reduce(red[:], t1[:], channels=128,
                                               reduce_op=bass_isa.ReduceOp.add)
                red2 = small_pool.tile([128, E], f32, name="red2", tag="red2")
                nc.vector.tensor_add(out=red2[:], in0=red[:], in1=battb_sb[:])
                nc.scalar.activation(out=scal_sb[:, b * E:(b + 1) * E],
                                     in_=red2[:], func=Act.Sigmoid)

            def emit_mix_chunk(b, c):
                """Mix chunk c of sample b's kernel, all bf16:
                kerT[c][cin, s*128+m] = sum_e r_be * convsT[e][c][cin, s*128+m]
                DVE: e0 tensor_scalar (4x mode) + e1..e5 STT chain.
                ScalarE: scaled copies of e6, e7 (activation scale=r).
                DVE folds: kt += (a6 + a7).
                """
                sc = lambda e: scal_sb[:, b * E + e:b * E + e + 1]
                a6 = amix_pool.tile([128, NSH * 128], f32r, name=f"a6_{c}",
                                    tag=f"a6_{c}")
                nc.scalar.activation(out=a6[:], in_=convsT_sb[6][c][:],
                                     func=Act.Copy, scale=sc(6))
                a7 = amix_pool.tile([128, NSH * 128], f32r, name=f"a7_{c}",
                                    tag=f"a7_{c}")
                nc.scalar.activation(out=a7[:], in_=convsT_sb[7][c][:],
                                     func=Act.Copy, scale=sc(7))
                k = kt_pool.tile([128, NSH * 128], f32r, name=f"kt{c}", tag=f"kt{c}")
                nc.vector.tensor_scalar_mul(out=k[:], in0=convsT_sb[0][c][:],
                                            scalar1=sc(0))
                for e in range(1, 6):
                    nc.vector.scalar_tensor_tensor(
                        out=k[:], in0=convsT_sb[e][c][:], scalar=sc(e),
                        in1=k[:], op0=Alu.mult, op1=Alu.add)
                nc.vector.tensor_add(out=k[:], in0=k[:], in1=a6[:])
                nc.vector.tensor_add(out=k[:], in0=k[:], in1=a7[:])
                return k

            def emit_route_mix(b, gs):
                emit_routing(b, gs)
                return [emit_mix_chunk(b, c) for c in range(CHUNKS)]

            def emit_conv(b, xp, kt):
                """Conv for sample b: accumulate 2c*9shift into 7 PSUM tiles.
                First and last passes + drains run in reverse tile order so
                PSUM banks recycle in consumption order (see module doc)."""
                cps = [cps_pool.tile([128, NFREE], f32, name=f"cps{n}",
                                     tag=f"cps{n}", bufs=2 if n == 0 else 1)
                       for n in range(NTILES)]
                for c in range(CHUNKS):
                    x3 = xp[c].rearrange("p (r q) -> p r q", q=WP)
                    for s in range(NSH):
                        dh, dw = s // KK, s % KK
                        lhsT = kt[c][:, s * 128:(s + 1) * 128]
                        first = (c == 0 and s == 0)
                        last = (c == CHUNKS - 1 and s == NSH - 1)
                        order = (range(NTILES - 1, -1, -1) if (first or last)
                                 else range(NTILES))
                        for n in order:
                            rhs = x3[:, n * ROWS_PER_TILE + dh:
                                     n * ROWS_PER_TILE + dh + ROWS_PER_TILE,
                                     dw:dw + W]
                            nc.tensor.matmul(cps[n][:], lhsT, rhs,
                                             start=first, stop=last)
                # drains in reverse order, alternating DVE/ScalarE; out DMAs
                # alternate SP/Activation rings (bf16 stores)
                for n in range(NTILES - 1, -1, -1):
                    o = out_pool.tile([128, NFREE], bf16, name="osb", tag="osb")
                    if n % 2 == 0:
                        nc.vector.tensor_copy(out=o[:], in_=cps[n][:])
                        deng = nc.sync
                    else:
                        nc.scalar.activation(out=o[:], in_=cps[n][:], func=Act.Copy)
                        deng = nc.scalar
                    deng.dma_start(
                        out=out_d[b, :, n * ROWS_PER_TILE:(n + 1) * ROWS_PER_TILE, :],
                        in_=o[:])

            # ---- software-pipelined emission ------------------------------
            S = SAMPLES_PER_CORE

            def emit_bank_chunk(c):
                """Expert bank chunk, tiles alternating SP/Activation rings."""
                for e in range(E):
                    t = res_pool.tile([128, NSH * 128], bf16,
                                      name=f"cv_{e}_{c}", tag=f"cv_{e}_{c}")
                    eng = nc.sync if e % 2 == 0 else nc.scalar
                    eng.dma_start(out=t[:], in_=convsT_d[e, c])
                    convsT_sb[e][c] = t

            loads = {0: emit_load(0, split4=True)}
            emit_bank_chunk(0)
            emit_routing(0, loads[0][1])
            kt0c0 = emit_mix_chunk(0, 0)
            emit_bank_chunk(1)
            kt0c1 = emit_mix_chunk(0, 1)
            kts = {0: [kt0c0, kt0c1]}
            loads[1] = emit_load(1, split4=True)
            emit_routing(1, loads[1][1])
            kt1c0 = emit_mix_chunk(1, 0)
            loads[2] = emit_load(2)
            emit_conv(0, loads.pop(0)[0], kts.pop(0))
            kts[1] = [kt1c0, emit_mix_chunk(1, 1)]
            for b in range(1, S):
                if b + 1 < S and b + 1 not in kts:
                    kts[b + 1] = emit_route_mix(b + 1, loads[b + 1][1])
                if b + 2 < S:
                    loads[b + 2] = emit_load(b + 2)
                emit_conv(b, loads.pop(b)[0], kts.pop(b))

    _dedup_ldweights(nc)
    nc.compile()
    return nc


def _prep_core_inputs(x, convs, W_att, b_att):
    """Host-side shard/layout prep. Returns list of 8 per-core input dicts."""
    import ml_dtypes
    f32 = np.float32
    bf16 = ml_dtypes.bfloat16
    # padded bf16 input, cin split into 2 chunks of 128
    xpad = np.zeros((B, CHUNKS, 128, HP, WP), dtype=f32)
    xpad[:, :, :, 1:H + 1, 1:W + 1] = np.ascontiguousarray(x, dtype=f32).reshape(
        B, CHUNKS, 128, H, W)
    xpad = xpad.reshape(B, CHUNKS, 128, PHW)

    # convsT[half][e, c, cin, s*128 + m] = convs[e, half*128+m, c*128+cin, kh, kw]
    cv = np.ascontiguousarray(convs, dtype=f32).reshape(E, 2, MHALF, CHUNKS, 128, NSH)
    convsT_halves = [
        np.ascontiguousarray(cv[:, h].transpose(0, 2, 3, 4, 1).reshape(
            E, CHUNKS, 128, NSH * 128)).astype(bf16)
        for h in range(2)
    ]

    watt = np.ascontiguousarray(
        (np.asarray(W_att, dtype=f32).T / f32(H * W)).reshape(CHUNKS, 128, E))
    battb = np.ascontiguousarray(
        np.broadcast_to(np.asarray(b_att, dtype=f32), (128, E)))

    in_maps = []
    for k in range(NCORES):
        pair, half = k // 2, k % 2
        sl = slice(pair * SAMPLES_PER_CORE, (pair + 1) * SAMPLES_PER_CORE)
        in_maps.append({
            "xpad": np.ascontiguousarray(xpad[sl]),
            "convsT": convsT_halves[half],
            "watt": watt,
            "battb": battb,
        })
    return in_maps


def _assemble_output(results):
    out = np.empty((B, COUT, H, W), dtype=np.float32)
    for k in range(NCORES):
        pair, half = k // 2, k % 2
        sl = slice(pair * SAMPLES_PER_CORE, (pair + 1) * SAMPLES_PER_CORE)
        out[sl, half * MHALF:(half + 1) * MHALF] = np.asarray(
            results[k]["out"], dtype=np.float32)
    return out


def kernel(x, convs, W_att, b_att):
    from concourse.bass_utils import run_bass_kernel_spmd

    if "nc" not in _cached:
        _cached["nc"] = _build_program()
    in_maps = _prep_core_inputs(x, convs, W_att, b_att)
    res = run_bass_kernel_spmd(_cached["nc"], in_maps, core_ids=list(range(NCORES)))
    return _assemble_output(res.results)
